# revision 1
# baseline (speedup 1.0000x reference)
"""BSDE solver kernel for Trainium2 (8 NeuronCores, data-parallel over paths).

Math (per path):
  S_t follows GBM: S_{t+1} = S_t * u_t,  u_t = 1 + R*DT + sigma*dw_t  (autonomous)
  Y_50 = c1^50 * Y0 + sum_t c1^(49-t) * zeta_t * sigma * S_t * dw_t,  c1 = 1+R*DT
  zeta_t = sigmoid(MLP(S_t/S0, t_t)) evaluated at B*50 independent points.

So the 50-step recurrence collapses to: bulk elementwise precompute (S-hat
cumulative product, v-tilde weights), one giant batched MLP over 1.6M points
(feature-major on the tensor engine, LayerNorm via weight-centering + Newton
rsqrt), and a weighted reduction.

Layout: a wave = 16 blocks x 400 points.  The MLP inputs for a whole wave live
in ONE [128, 400] tile (row = 32*chunk + 16*feature + block), gathered from
the batch-major S-hat/t tiles with eight [16,400] contiguous-row DMAs (DMA
cost here is ~0.39ns per row-BYTE, row count nearly free — so shuffles must be
many-short-rows, never one long row).  Each per-block matmul is a single
full-K instruction whose stationary is a 128-wide column slice of a
16-variant constant (variant = block): L1 input slabs, rstd-broadcast
selectors (reading the Newton-rsqrt output tile directly as the moving
operand — no rstd staging copies at all), and sliding one-hot variance/L3
slabs that accumulate a whole wave into one [64,400] PSUM stack
(row = 32*pair + 16*j + block).  Zeta returns to batch-major as bf16 with
four [4,1600]<-[16,400] scatters per wave.

LayerNorm trick: weights are column-centered on the host so z has exactly
zero feature-mean; gamma folds into the weights, the variance matmul uses
1/gamma^2, beta rides the Gelu bias.  Sigmoid = 0.5*(1+tanh(x/2)) keeps all
ACT functions in one table set.  Datapath is bf16 (1 cyc/row matmuls; rel-err
budget 2e-2, bf16 lands ~6e-4).  GPSIMD/Pool cannot access PSUM on real HW,
so PSUM-side work (evictions, LN applies) is split between ACT and DVE —
z lives in 2-bank [128,1024] PSUM tiles whose two 400-col halves are
processed by single strided-AP ops — while Pool takes the SBUF-only squares
and the Newton-rsqrt tail.  Emission is stage-skewed across blocks and
loop3(w) interleaves with loop1(w+1) to hide the rsqrt barriers.
"""

import sys

sys.path.insert(0, "/opt/trn_rl_repo")

import numpy as np

import concourse.bass as bass
import concourse.bacc as bacc
import concourse.tile as tile
import concourse.mybir as mybir
import concourse.bass_utils as bass_utils

F32 = mybir.dt.float32
MMDT = mybir.dt.bfloat16  # MLP datapath dtype
I32 = mybir.dt.int32
ALU = mybir.AluOpType
ACTF = mybir.ActivationFunctionType

# Problem constants (hardcoded per spec).
B, MSTEPS, H = 32768, 50, 64
S0, R, SIGMA = 100.0, 0.05, 0.2
DT = 1.0 / MSTEPS
C1 = 1.0 + R * DT
EPS = 1e-5
NCORES = 8
NT = 400  # block free-size (divides the 50*G per-partition segment)
WAVE = 32  # blocks per wave: stack rows 64*pair+32*j+bi fill all 128 partitions
MAGIC = 0x5F3759DF
NR_ITERS = 1  # magic-guess + 1 Newton step: rstd rel err ~1.7e-3, under bf16 noise
SLC = 95  # slab marker column: slab slice for (pair,bi) starts at SLC-64*pair-bi


def _prep_weights(ins):
    """Host-side weight preprocessing (tiny, O(H^2))."""

    def prep(W, b, g):
        Wc = (W.astype(np.float64) - W.astype(np.float64).mean(axis=1, keepdims=True))
        bc = b.astype(np.float64)
        bc = bc - bc.mean()
        return (Wc * g[None, :]).astype(np.float32), (bc * g).astype(np.float32)

    W1g, b1g = prep(ins["W1"], ins["b1"], ins["g1"])
    W2g, b2g = prep(ins["W2"], ins["b2"], ins["g2"])
    ig1 = (1.0 / ins["g1"].astype(np.float64) ** 2).astype(np.float32)
    ig2 = (1.0 / ins["g2"].astype(np.float64) ** 2).astype(np.float32)

    d = {}
    # L1 stationary, 16 variants (one per block in a wave): within each
    # 64-row pair half, moving row 32*c2 + 16*q + bi holds input feature q of
    # chunk c2 for block bi -> maps to W1g[q] at output cols [64*c2, 64*c2+64)
    w14v = np.zeros((128, 16 * 128), np.float32)
    for ph in (0, 64):
        for bi in range(16):
            for c2 in (0, 1):
                for q in (0, 1):
                    w14v[ph + 32 * c2 + 16 * q + bi,
                         128 * bi + 64 * c2 : 128 * bi + 64 * c2 + 64] = W1g[q]
    d["w14v"] = w14v
    w22bd = np.zeros((128, 128), np.float32)
    w22bd[0:64, 0:64] = W2g
    w22bd[64:128, 64:128] = W2g
    d["w22bd"] = w22bd
    # rstd-broadcast selector, 32 variants: out rows [0,64) take stack row
    # 64p+bi (j=0), rows [64,128) take 64p+32+bi (j=1); K=64 moving = the
    # Newton-rsqrt output rows [64p, 64p+64) read in place.
    selc = np.zeros((128, 32 * 128), np.float32)
    for ph in (0, 64):
        for bi in range(32):
            selc[ph + bi, 128 * bi : 128 * bi + 64] = 1.0
            selc[ph + 32 + bi, 128 * bi + 64 : 128 * bi + 128] = 1.0
    d["selc"] = selc
    w3 = ins["W3"].reshape(H).astype(np.float32)

    def slab(vec):
        s = np.zeros((128, 224), np.float32)
        s[0:64, SLC] = vec
        s[64:128, SLC + 32] = vec
        return s

    d["igsl1"] = slab(ig1)
    d["igsl2"] = slab(ig2)
    d["w3sl"] = slab(w3)
    d["b1c"] = np.tile(b1g.reshape(H, 1), (2, 1))
    d["b2c"] = np.tile(b2g.reshape(H, 1), (2, 1))
    d["be1c"] = np.tile(ins["be1"].reshape(H, 1).astype(np.float32), (2, 1))
    d["be2c"] = np.tile(ins["be2"].reshape(H, 1).astype(np.float32), (2, 1))
    d["b3h"] = np.full((128, 1), 0.5 * float(ins["b3"][0]), np.float32)
    d["y0c"] = np.full((128, 1), (C1**MSTEPS) * float(ins["Y0"][0]), np.float32)
    return d


def _afull(G):
    A = (C1 ** (MSTEPS - 1 - np.arange(MSTEPS)) * SIGMA * S0).astype(np.float32)
    return np.tile(A.reshape(1, 1, MSTEPS), (128, G, 1)).reshape(128, G * MSTEPS)


CONS_SPECS = {
    "w14v": [128, 2048], "w22bd": [128, 128], "selc": [128, 4096],
    "igsl1": [128, 224], "igsl2": [128, 224], "w3sl": [128, 224],
    "b1c": [128, 1], "b2c": [128, 1], "be1c": [128, 1],
    "be2c": [128, 1], "b3h": [128, 1], "y0c": [128, 1],
}
MMCONS = ("w14v", "w22bd", "selc", "igsl1", "igsl2", "w3sl")


def build_program(G=32, gelu=ACTF.Gelu):
    """Build the per-core Bass program. G = path-groups per partition (BC=128*G)."""
    BC = 128 * G
    SEG = G * MSTEPS  # per-partition fm segment length
    PB = SEG // NT  # blocks per partition-segment
    assert SEG % NT == 0
    NBLK = 32 * SEG // NT  # col-blocks (each spans all 4 chunk-rows)
    assert NBLK % WAVE == 0
    HWV = WAVE // PB  # partitions-worth of blocks per wave
    assert WAVE == HWV * PB

    nc = bacc.Bacc("TRN2", target_bir_lowering=False, debug=False, num_devices=NCORES)

    dw_d = nc.dram_tensor("dw", [BC, MSTEPS], F32, kind="ExternalInput")
    tg_d = nc.dram_tensor("tg", [BC, MSTEPS], F32, kind="ExternalInput")
    cons_d = {k: nc.dram_tensor(k, s, F32, kind="ExternalInput") for k, s in CONS_SPECS.items()}
    af_d = nc.dram_tensor("afull", [128, SEG], F32, kind="ExternalInput")
    yo_d = nc.dram_tensor("yo", [BC, 1], F32, kind="ExternalOutput")
    so_d = nc.dram_tensor("so", [BC, 1], F32, kind="ExternalOutput")

    with tile.TileContext(nc) as tc:
        with (
            tc.tile_pool(name="cons", bufs=1) as cpool,
            tc.tile_pool(name="bm", bufs=1) as bmpool,
            tc.tile_pool(name="x4", bufs=4) as x4pool,
            tc.tile_pool(name="zs", bufs=66) as zspool,
            tc.tile_pool(name="q", bufs=3) as qpool,
            tc.tile_pool(name="h", bufs=4) as hpool,
            tc.tile_pool(name="nr", bufs=2) as nrpool,
            tc.tile_pool(name="nri", bufs=2) as nripool,
            tc.tile_pool(name="zc", bufs=2) as zcpool,
            tc.tile_pool(name="scr", bufs=2) as scrpool,
            tc.tile_pool(name="zp", bufs=3, space="PSUM") as zppool,
            tc.tile_pool(name="ssp", bufs=2, space="PSUM") as sspool,
        ):
            # ---- load constants ----
            cons, consr = {}, {}
            for k, s in CONS_SPECS.items():
                if k in MMCONS:
                    continue
                t = cpool.tile(s, F32, tag=k)
                nc.sync.dma_start(t[:], cons_d[k].ap())
                cons[k] = t
            # bf16 copies of matmul operand consts (staged via scratch)
            for k in MMCONS:
                s = CONS_SPECS[k]
                tr = cpool.tile(s, MMDT, tag=k + "r")
                for c0 in range(0, s[1], 2048):
                    cw = min(2048, s[1] - c0)
                    stg = scrpool.tile([128, 2048], F32, tag="scr")
                    nc.sync.dma_start(stg[: s[0], :cw], cons_d[k].ap()[:, c0 : c0 + cw])
                    nc.scalar.activation(tr[:, c0 : c0 + cw], stg[: s[0], :cw], ACTF.Identity)
                consr[k] = tr
            af = cpool.tile([128, SEG], F32, tag="afull")
            nc.sync.dma_start(af[:], af_d.ap())

            # ---- phase A: batch-major precompute ----
            dwb = bmpool.tile([128, SEG], F32, tag="dwb")
            nc.sync.dma_start(dwb[:], dw_d.ap().rearrange("(p g) t -> p (g t)", p=128))
            u = bmpool.tile([128, SEG], F32, tag="u")
            nc.gpsimd.tensor_scalar(u[:], dwb[:], SIGMA, 1.0 + R * DT, ALU.mult, ALU.add)
            sh = bmpool.tile([128, SEG], F32, tag="sh")
            nc.vector.memset(sh[:], 1.0)
            sh3 = sh[:].rearrange("p (g t) -> p g t", t=MSTEPS)
            u3 = u[:].rearrange("p (g t) -> p g t", t=MSTEPS)
            for t in range(1, MSTEPS):
                nc.vector.tensor_tensor(sh3[:, :, t], sh3[:, :, t - 1], u3[:, :, t - 1], ALU.mult)
            vt = bmpool.tile([128, SEG], F32, tag="vt")
            nc.gpsimd.tensor_tensor(vt[:], dwb[:], af[:], ALU.mult)
            nc.gpsimd.tensor_tensor(vt[:], vt[:], sh[:], ALU.mult)
            sout = bmpool.tile([128, G], F32, tag="sout")
            nc.vector.scalar_tensor_tensor(
                sout[:], sh3[:, :, MSTEPS - 1], S0, u3[:, :, MSTEPS - 1], ALU.mult, ALU.mult
            )
            nc.sync.dma_start(so_d.ap().rearrange("(p g) o -> p (g o)", p=128), sout[:])

            zb = bmpool.tile([128, SEG], MMDT, tag="zb")
            # bf16 copies of Shat and t for MLP inputs
            shr = bmpool.tile([128, SEG], MMDT, tag="shr")
            nc.scalar.activation(shr[:], sh[:], ACTF.Identity)
            tgb = scrpool.tile([128, 2048], F32, tag="scr")
            nc.sync.dma_start(tgb[:, :SEG], tg_d.ap().rearrange("(p g) t -> p (g t)", p=128))
            tgr = bmpool.tile([128, SEG], MMDT, tag="tgr")
            nc.scalar.activation(tgr[:], tgb[:, :SEG], ACTF.Identity)

            def nr_rsqrt(sstk, eng):
                """Newton rsqrt of mean(sstk)/H over 64 stack rows (EPS
                dropped: variance >> 1e-5 here).  rsqrt(s/64) = 8*rsqrt(s):
                Newton runs on s directly; the final step's constants fold
                in the *8.  PSUM reads go to DVE (shift) and ACT (staging
                copy, in parallel); the SBUF tail runs on `eng` (Pool)."""
                NR = 4 * WAVE
                sh1 = nripool.tile([128, NT], I32, tag="sh1")
                nc.vector.tensor_scalar(
                    sh1[:NR, :], sstk[:NR, :].bitcast(I32), 1, None, ALU.logical_shift_right
                )
                vh = nrpool.tile([128, NT], F32, tag="vh")
                nc.scalar.activation(vh[:NR, :], sstk[:NR, :], ACTF.Identity)
                y = nrpool.tile([128, NT], F32, tag="ynr")
                eng.tensor_scalar(
                    y[:NR, :].bitcast(I32), sh1[:NR, :], -1, MAGIC, ALU.mult, ALU.add
                )
                ta = nrpool.tile([128, NT], F32, tag="ta")
                yr = nrpool.tile([128, NT], MMDT, tag="yr")
                eng.tensor_tensor(ta[:NR, :], y[:NR, :], y[:NR, :], ALU.mult)
                eng.tensor_tensor(ta[:NR, :], ta[:NR, :], vh[:NR, :], ALU.mult)
                eng.tensor_scalar(ta[:NR, :], ta[:NR, :], -4.0, 12.0, ALU.mult, ALU.add)
                eng.tensor_tensor(yr[:NR, :], y[:NR, :], ta[:NR, :], ALU.mult)
                return yr

            # ---- phase B: waves of 16 blocks ----
            # Wave-input tile x4v [128, NT]: row = 32*chunk + 16*q + bi.
            # Stack row for (pair p, j, block bi) = 32p + 16j + bi.
            shv = shr[:].rearrange("p (l c) -> p l c", l=PB)
            tgv = tgr[:].rearrange("p (l c) -> p l c", l=PB)

            def slab_mm(stk, bi, p, slabc, rhs, first, last):
                nc.tensor.matmul(
                    stk[0:128, :],
                    slabc[:, SLC - 64 * p - bi : SLC - 64 * p - bi + 128],
                    rhs,
                    start=first, stop=last,
                    tile_position=(0, 0),
                    skip_group_check=True,
                )

            NW = NBLK // WAVE

            def gather(w):
                halves = []
                for hh in (0, 1):
                    x4v = x4pool.tile([128, NT], MMDT, tag="x4v", name=f"x4v{w}_{hh}")
                    for i in range(4):
                        p0 = 32 * i + HWV * w + 4 * hh
                        nc.sync.dma_start(x4v[32 * i : 32 * i + 16, :], shv[p0 : p0 + 4])
                        nc.sync.dma_start(x4v[32 * i + 16 : 32 * i + 32, :], tgv[p0 : p0 + 4])
                    halves.append(x4v)
                return halves

            # Stage-skewed emission: per-engine queues are in-order, so each
            # k-iteration interleaves independent stages of consecutive
            # blocks (block k's input matmul, block k-1's elementwise, block
            # k-2's stack matmul).  Additionally loop3 of wave w is emitted
            # interleaved with loop1 of wave w+1 (complementary engine
            # profiles, and it hides both Newton-rsqrt barriers).
            def pv(t):
                # [128, 2, NT] strided view of a 2-bank psum tile: the two
                # 400-col halves at bank offsets 0 and 512
                return t[:].rearrange("p (u c) -> p u c", u=2)[:, :, :NT]

            def sv2(t):
                return t[:].rearrange("p (u c) -> p u c", u=2)

            def a_mm(st, bi):
                zp1 = zppool.tile([128, 1024], F32, tag="zp", name=f"zp1_{bi}")
                x4h = st["x4v"][bi // 16]
                bl = bi % 16
                for p in (0, 1):
                    nc.tensor.matmul(
                        zp1[:, 512 * p : 512 * p + NT],
                        consr["w14v"][64 * p : 64 * p + 64, 128 * bl : 128 * bl + 128],
                        x4h[64 * p : 64 * p + 64, :],
                        start=True, stop=True, tile_position=(64 * p, 0),
                    )
                st["zp1"][bi] = zp1

            def a_ev(st, bi):
                zp1 = st["zp1"][bi]
                zs = zspool.tile([128, 2 * NT], MMDT, tag="zs", name=f"zs1_{bi}")
                nc.scalar.activation(zs[:, :NT], zp1[:, :NT], ACTF.Identity, bias=cons["b1c"][:])
                nc.vector.tensor_scalar(zs[:, NT:], zp1[:, 512 : 512 + NT], cons["b1c"][:], None, ALU.add)
                st["zs1"][bi] = zs

            def a_q(st, bi):
                q = qpool.tile([128, 2 * NT], MMDT, tag="q", name=f"q1_{bi}")
                nc.gpsimd.tensor_tensor(q[:], st["zs1"][bi][:], st["zs1"][bi][:], ALU.mult)
                st["q1"][bi] = q

            def a_ss(st, bi):
                for p in (0, 1):
                    slab_mm(st["sstk1"], bi, p, consr["igsl1"],
                            st["q1"][bi][:][:, p * NT : p * NT + NT],
                            first=(bi == 0 and p == 0), last=(bi == WAVE - 1 and p == 1))

            def b_rb(st, bi):
                rb1 = zppool.tile([128, 1024], F32, tag="zp", name=f"rb1_{bi}")
                for p in (0, 1):
                    nc.tensor.matmul(
                        rb1[:, 512 * p : 512 * p + NT],
                        consr["selc"][64 * p : 64 * p + 64, 128 * bi : 128 * bi + 128],
                        st["rstd1"][64 * p : 64 * p + 64, :],
                        start=True, stop=True, tile_position=(64 * p, 0),
                    )
                st["rb1"][bi] = rb1

            def b_ew(st, bi):
                rb1 = st["rb1"][bi]
                zs = st["zs1"][bi]
                nc.vector.tensor_tensor(sv2(zs), sv2(zs), pv(rb1), ALU.mult)
                h1 = hpool.tile([128, 2 * NT], MMDT, tag="h", name=f"h1_{bi}")
                nc.scalar.activation(h1[:], zs[:], gelu, bias=cons["be1c"][:])
                zp2 = zppool.tile([128, 1024], F32, tag="zp", name=f"zp2_{bi}")
                for p in (0, 1):
                    nc.tensor.matmul(
                        zp2[:, 512 * p : 512 * p + NT],
                        consr["w22bd"][:, :],
                        h1[:, p * NT : p * NT + NT],
                        start=True, stop=True, tile_position=(0, 0),
                    )
                st["zp2"][bi] = zp2

            def b_ev(st, bi):
                zp2 = st["zp2"][bi]
                zs_2 = zspool.tile([128, 2 * NT], MMDT, tag="zs", name=f"zs2_{bi}")
                nc.vector.tensor_scalar(zs_2[:, :NT], zp2[:, :NT], cons["b2c"][:], None, ALU.add)
                nc.scalar.activation(zs_2[:, NT:], zp2[:, 512 : 512 + NT], ACTF.Identity, bias=cons["b2c"][:])
                q = qpool.tile([128, 2 * NT], MMDT, tag="q", name=f"q2_{bi}")
                nc.gpsimd.tensor_tensor(q[:], zs_2[:], zs_2[:], ALU.mult)
                st["zs2"][bi], st["q2"][bi] = zs_2, q

            def b_ss(st, bi):
                for p in (0, 1):
                    slab_mm(st["sstk2"], bi, p, consr["igsl2"],
                            st["q2"][bi][:][:, p * NT : p * NT + NT],
                            first=(bi == 0 and p == 0), last=(bi == WAVE - 1 and p == 1))

            def c_rb(st, bi):
                rb2 = zppool.tile([128, 1024], F32, tag="zp", name=f"rb2_{bi}")
                for p in (0, 1):
                    nc.tensor.matmul(
                        rb2[:, 512 * p : 512 * p + NT],
                        consr["selc"][64 * p : 64 * p + 64, 128 * bi : 128 * bi + 128],
                        st["rstd2"][64 * p : 64 * p + 64, :],
                        start=True, stop=True, tile_position=(64 * p, 0),
                    )
                st["rb2"][bi] = rb2

            def c_ew(st, bi):
                rb2 = st["rb2"][bi]
                zs_2 = st["zs2"][bi]
                nc.vector.tensor_tensor(sv2(zs_2), sv2(zs_2), pv(rb2), ALU.mult)
                h2 = hpool.tile([128, 2 * NT], MMDT, tag="h", name=f"h2_{bi}")
                nc.scalar.activation(h2[:], zs_2[:], gelu, bias=cons["be2c"][:])
                st["h2"][bi] = h2

            def c_ss(st, bi):
                for p in (0, 1):
                    slab_mm(st["zstk"], bi, p, consr["w3sl"],
                            st["h2"][bi][:][:, p * NT : p * NT + NT],
                            first=(bi == 0 and p == 0), last=(bi == WAVE - 1 and p == 1))

            def emit_A(st):
                st["sstk1"] = sspool.tile([128, NT], F32, tag="stk", name="sstk1")
                for k in range(WAVE + 3):
                    if k < WAVE:
                        a_mm(st, k)
                    if 0 <= k - 1 < WAVE:
                        a_ev(st, k - 1)
                    if 0 <= k - 2 < WAVE:
                        a_q(st, k - 2)
                    if 0 <= k - 3 < WAVE:
                        a_ss(st, k - 3)
                st["rstd1"] = nr_rsqrt(st["sstk1"], nc.gpsimd)

            def new_state(w):
                return {"w": w, "x4v": gather(w),
                        "zp1": [None] * WAVE, "zs1": [None] * WAVE, "q1": [None] * WAVE,
                        "rb1": [None] * WAVE, "zp2": [None] * WAVE,
                        "zs2": [None] * WAVE, "q2": [None] * WAVE,
                        "rb2": [None] * WAVE, "h2": [None] * WAVE}

            st = new_state(0)
            emit_A(st)
            for w in range(NW):
                # -- B(w): rstd bcast; LN1 apply; gelu -> h1; L2; evict; sq; SS2 --
                st["sstk2"] = sspool.tile([128, NT], F32, tag="stk", name="sstk2")
                for k in range(WAVE + 3):
                    if k < WAVE:
                        b_rb(st, k)
                    if 0 <= k - 1 < WAVE:
                        b_ew(st, k - 1)
                    if 0 <= k - 2 < WAVE:
                        b_ev(st, k - 2)
                    if 0 <= k - 3 < WAVE:
                        b_ss(st, k - 3)
                st["rstd2"] = nr_rsqrt(st["sstk2"], nc.gpsimd)

                # -- C(w) interleaved with A(w+1) --
                st["zstk"] = sspool.tile([128, NT], F32, tag="stk", name="zstk")
                stn = new_state(w + 1) if w + 1 < NW else None
                if stn is not None:
                    stn["sstk1"] = sspool.tile([128, NT], F32, tag="stk", name="sstk1")
                for k in range(WAVE + 3):
                    if k < WAVE:
                        c_rb(st, k)
                        if stn is not None:
                            a_mm(stn, k)
                    if 0 <= k - 1 < WAVE:
                        c_ew(st, k - 1)
                        if stn is not None:
                            a_ev(stn, k - 1)
                    if 0 <= k - 2 < WAVE:
                        c_ss(st, k - 2)
                        if stn is not None:
                            a_q(stn, k - 2)
                    if 0 <= k - 3 < WAVE and stn is not None:
                        a_ss(stn, k - 3)
                # zeta rows psum->sbuf (bf16), then 4 contiguous-row scatters
                zsc = zcpool.tile([128, NT], MMDT, tag="zsc")
                nc.scalar.activation(zsc[:, :], st["zstk"][:, :], ACTF.Identity)
                for i in range(4):
                    a_, b_ = i // 2, i % 2
                    nc.sync.dma_start(
                        zb[32 * i + HWV * w : 32 * i + HWV * w + HWV, :],
                        zsc[64 * a_ + 32 * b_ : 64 * a_ + 32 * b_ + 32, :],
                    )
                if stn is not None:
                    stn["rstd1"] = nr_rsqrt(stn["sstk1"], nc.gpsimd)
                st = stn

            # ---- phase C: zeta -> Y ----
            tbm = bmpool.tile([128, SEG], F32, tag="dwb")
            nc.scalar.activation(tbm[:], zb[:], ACTF.Tanh, bias=cons["b3h"][:], scale=0.5)
            nc.vector.scalar_tensor_tensor(tbm[:], tbm[:], 1.0, vt[:], ALU.add, ALU.mult)
            ps = bmpool.tile([128, G], F32, tag="ps")
            nc.vector.tensor_reduce(
                ps[:], tbm[:].rearrange("p (g t) -> p g t", t=MSTEPS), mybir.AxisListType.X, ALU.add
            )
            yout = bmpool.tile([128, G], F32, tag="yout")
            nc.vector.tensor_scalar(yout[:], ps[:], 0.5, cons["y0c"][:], ALU.mult, ALU.add)
            nc.sync.dma_start(yo_d.ap().rearrange("(p g) o -> p (g o)", p=128), yout[:])

    nc.compile()
    return nc


_CACHE = {}


def _get_program(G=32):
    if G not in _CACHE:
        _CACHE[G] = build_program(G)
    return _CACHE[G]


def make_in_maps(inputs, G=32):
    BC = 128 * G
    cons = _prep_weights(inputs)
    cons["afull"] = _afull(G)
    dw = np.ascontiguousarray(np.asarray(inputs["dw"], np.float32)[: NCORES * BC])
    tg = np.ascontiguousarray(np.asarray(inputs["t_grid"], np.float32)[: NCORES * BC])
    maps = []
    for c in range(NCORES):
        m = {"dw": dw[c * BC : (c + 1) * BC], "tg": tg[c * BC : (c + 1) * BC]}
        m.update(cons)
        maps.append(m)
    return maps


def kernel(**inputs):
    nc = _get_program()
    in_maps = make_in_maps(inputs)
    res = bass_utils.run_bass_kernel_spmd(nc, in_maps, core_ids=list(range(NCORES)))
    Y = np.concatenate([res.results[c]["yo"] for c in range(NCORES)], axis=0)
    S = np.concatenate([res.results[c]["so"] for c in range(NCORES)], axis=0)
    return Y.reshape(B, 1).astype(np.float32), S.reshape(B, 1).astype(np.float32)

